# revision 9
# baseline (speedup 1.0000x reference)
import os
import sys
import numpy as np

sys.path.insert(0, "/opt/trn_rl_repo")

# Problem constants (hardcoded per spec: B=2, T=4096, H=32, C=64)
B, T, H, C = 2, 4096, 32, 64
BH = B * H          # 64 (b,h) slices
NCORES = 8
NH = BH // NCORES   # 8 heads per core
DT = 32             # chunk length used on device (math-equivalent to ref DT=16)
BLK = 128           # tokens per processed block (4 chunks)
CH = BLK // DT      # chunks per block = 4
NB = T // BLK       # 32 blocks per head
NT = T // DT        # 128 chunks per head

_CACHED = {}


def _build_masks():
    t = np.arange(BLK)
    same = (t[:, None] // DT) == (t[None, :] // DT)
    # maskp[s, 0:128]   -> m2T: s<=t within chunk (for qkT, qbT)
    # maskp[s, 128:256] -> m1T: s<t  within chunk (for akT, abT)
    m2T = ((t[:, None] <= t[None, :]) & same).astype(np.float32)
    m1T = ((t[:, None] < t[None, :]) & same).astype(np.float32)
    m1 = ((t[:, None] > t[None, :]) & same).astype(np.float32)  # for ab
    maskp = np.concatenate([m2T, m1T], axis=1)
    return maskp, m1


def _build_nc(NHb=NH, NBb=NB, Tb=T):
    import concourse.bass as bass
    import concourse.bacc as bacc
    import concourse.mybir as mybir
    from concourse.tile import TileContext

    dt = mybir.dt
    f32, bf16 = dt.float32, dt.bfloat16
    AO = mybir.AluOpType

    nc = bacc.Bacc("TRN2")
    NH_, NB_, T_ = NHb, NBb, Tb
    # c-major bf16 streams [NH_, C, T_]
    cwq = nc.dram_tensor("cwq", [NH_, C, T_], bf16, kind="ExternalInput")
    cwa = nc.dram_tensor("cwa", [NH_, C, T_], bf16, kind="ExternalInput")
    ckw = nc.dram_tensor("ckw", [NH_, C, T_], bf16, kind="ExternalInput")
    cbw = nc.dram_tensor("cbw", [NH_, C, T_], bf16, kind="ExternalInput")
    # time-major bf16 streams, pre-tiled [NH, 128, NB*C]
    tv = nc.dram_tensor("tv", [NH_, BLK, NB_ * C], bf16, kind="ExternalInput")
    twa = nc.dram_tensor("twa", [NH_, BLK, NB_ * C], bf16, kind="ExternalInput")
    tkwf = nc.dram_tensor("tkwf", [NH_, BLK, NB_ * C], bf16, kind="ExternalInput")
    tbwf = nc.dram_tensor("tbwf", [NH_, BLK, NB_ * C], bf16, kind="ExternalInput")
    # per-chunk whole-chunk decay [NH_, C, T_ // DT] fp32
    fwd = nc.dram_tensor("fwd", [NH_, C, T_ // DT], f32, kind="ExternalInput")
    # constants
    maskp = nc.dram_tensor("maskp", [BLK, 2 * BLK], bf16, kind="ExternalInput")
    m1c = nc.dram_tensor("m1c", [BLK, BLK], bf16, kind="ExternalInput")
    ident = nc.dram_tensor("ident", [BLK, BLK], bf16, kind="ExternalInput")
    # output, c-major fp32
    y = nc.dram_tensor("y", [NH_, C, T_], f32, kind="ExternalOutput")

    with TileContext(nc) as tc:
        with (
            tc.tile_pool(name="const", bufs=1) as constp,
            tc.tile_pool(name="cstream", bufs=2) as csp,
            tc.tile_pool(name="tstream", bufs=2) as tsp,
            tc.tile_pool(name="gram", bufs=3) as gp,
            tc.tile_pool(name="pow", bufs=3) as pp,
            tc.tile_pool(name="xapp", bufs=3) as xp,
            tc.tile_pool(name="state", bufs=2) as stp,
            tc.tile_pool(name="small", bufs=4) as smp,
            tc.tile_pool(name="yout", bufs=3) as yp,
            tc.tile_pool(name="ps", bufs=1, space="PSUM") as psp,
        ):
            mk = constp.tile([BLK, 2 * BLK], bf16, tag="mk")
            nc.sync.dma_start(mk[:], maskp[:])
            m1t = constp.tile([BLK, BLK], bf16, tag="m1t")
            nc.sync.dma_start(m1t[:], m1c[:])
            idt = constp.tile([BLK, BLK], bf16, tag="idt")
            nc.sync.dma_start(idt[:], ident[:])

            for h in range(NH_):
                # per-head stream tiles
                qa = csp.tile([C, 2 * T_], bf16, tag="qa")
                nc.sync.dma_start(qa[:, 0:T_], cwq[h])
                nc.sync.dma_start(qa[:, T_ : 2 * T_], cwa[h])
                ck = csp.tile([C, T_], bf16, tag="ck")
                nc.sync.dma_start(ck[:], ckw[h])
                cb = csp.tile([C, T_], bf16, tag="cb")
                nc.sync.dma_start(cb[:], cbw[h])
                tvt = tsp.tile([BLK, NB_ * C], bf16, tag="tv")
                nc.sync.dma_start(tvt[:], tv[h])
                twat = tsp.tile([BLK, NB_ * C], bf16, tag="twa")
                nc.sync.dma_start(twat[:], twa[h])
                tkft = tsp.tile([BLK, NB_ * C], bf16, tag="tkf")
                nc.sync.dma_start(tkft[:], tkwf[h])
                tbft = tsp.tile([BLK, NB_ * C], bf16, tag="tbf")
                nc.sync.dma_start(tbft[:], tbwf[h])
                fwt = smp.tile([C, T_ // DT], f32, tag="fw")
                nc.sync.dma_start(fwt[:], fwd[h])

                # state ping-pong [kc, vc] bf16
                sts = [
                    stp.tile([C, C], bf16, tag="st0", name=f"st0_{h}"),
                    stp.tile([C, C], bf16, tag="st1", name=f"st1_{h}"),
                ]
                nc.vector.memset(sts[0][:], 0.0)
                cur = 0

                qa3 = qa.rearrange("c (two t) -> c two t", two=2)

                for n in range(NB_):
                    t0 = n * BLK
                    tm = slice(n * C, (n + 1) * C)
                    # ---- Grams ----
                    p1 = psp.tile([BLK, 2 * BLK], f32, tag="pbig", bufs=3)
                    nc.tensor.matmul(
                        p1[:], ck[:, t0 : t0 + BLK], qa3[:, :, t0 : t0 + BLK], start=True, stop=True
                    )
                    p2 = psp.tile([BLK, 2 * BLK], f32, tag="pbig", bufs=3)
                    nc.tensor.matmul(
                        p2[:], cb[:, t0 : t0 + BLK], qa3[:, :, t0 : t0 + BLK], start=True, stop=True
                    )
                    p3 = psp.tile([BLK, BLK], f32, tag="pmid", bufs=3)
                    nc.tensor.matmul(
                        p3[:],
                        qa[:, T_ + t0 : T_ + t0 + BLK],
                        cb[:, t0 : t0 + BLK],
                        start=True,
                        stop=True,
                    )
                    g1 = gp.tile([BLK, 2 * BLK], bf16, tag="g1")  # [qkT_m | akT_m]
                    nc.vector.tensor_tensor(g1[:], p1[:], mk[:], op=AO.mult)
                    g2 = gp.tile([BLK, 2 * BLK], bf16, tag="g2")  # [qbT_m | abT_m]
                    nc.vector.tensor_tensor(g2[:], p2[:], mk[:], op=AO.mult)
                    g3 = gp.tile([BLK, BLK], bf16, tag="g3")  # ab_m
                    nc.vector.tensor_tensor(g3[:], p3[:], m1t[:], op=AO.mult)

                    # ---- u0 and Z = [u0 | wa_tm] ----
                    pz = psp.tile([BLK, C], f32, tag="pmid", bufs=3)
                    nc.tensor.matmul(
                        pz[:], g1[:, BLK : 2 * BLK], tvt[:, tm], start=True, stop=True
                    )
                    zx = xp.tile([BLK, 2 * C], bf16, tag="zx")
                    nc.scalar.copy(zx[:, 0:C], pz[:])
                    nc.vector.tensor_copy(zx[:, C : 2 * C], twat[:, tm])

                    # ---- Neumann powers (blockdiag DT=32): AT2..AT16 ----
                    abT = g2.rearrange("s (two t) -> s two t", two=2)[:, 1, :]
                    pw = psp.tile([BLK, BLK], f32, tag="pmid", bufs=3)
                    nc.tensor.matmul(pw[:], g3[:], abT, start=True, stop=True)
                    at2 = pp.tile([BLK, BLK], bf16, tag="at2")
                    nc.scalar.copy(at2[:], pw[:])
                    pw2 = psp.tile([BLK, BLK], f32, tag="pmid", bufs=3)
                    nc.tensor.matmul(pw2[:], abT, g3[:], start=True, stop=True)
                    a2 = pp.tile([BLK, BLK], bf16, tag="a2")
                    nc.scalar.copy(a2[:], pw2[:])
                    pw3 = psp.tile([BLK, BLK], f32, tag="pmid", bufs=3)
                    nc.tensor.matmul(pw3[:], a2[:], at2[:], start=True, stop=True)
                    at4 = pp.tile([BLK, BLK], bf16, tag="at4")
                    nc.scalar.copy(at4[:], pw3[:])
                    pw4 = psp.tile([BLK, BLK], f32, tag="pmid", bufs=3)
                    nc.tensor.matmul(pw4[:], at2[:], a2[:], start=True, stop=True)
                    a4 = pp.tile([BLK, BLK], bf16, tag="a4")
                    nc.scalar.copy(a4[:], pw4[:])
                    pw5 = psp.tile([BLK, BLK], f32, tag="pmid", bufs=3)
                    nc.tensor.matmul(pw5[:], a4[:], at4[:], start=True, stop=True)
                    at8 = pp.tile([BLK, BLK], bf16, tag="at8")
                    nc.scalar.copy(at8[:], pw5[:])
                    pw6 = psp.tile([BLK, BLK], f32, tag="pmid", bufs=3)
                    nc.tensor.matmul(pw6[:], at4[:], a4[:], start=True, stop=True)
                    a8 = pp.tile([BLK, BLK], bf16, tag="a8")
                    nc.scalar.copy(a8[:], pw6[:])
                    pw7 = psp.tile([BLK, BLK], f32, tag="pmid", bufs=3)
                    nc.tensor.matmul(pw7[:], a8[:], at8[:], start=True, stop=True)
                    at16 = pp.tile([BLK, BLK], bf16, tag="at16")
                    nc.scalar.copy(at16[:], pw7[:])

                    # ---- Neumann applies: X = (I+AT^p)... chain on Z ----
                    xcur = zx
                    for atp in (abT, at2[:], at4[:], at8[:], at16[:]):
                        px = psp.tile([BLK, 2 * C], f32, tag="pmid", bufs=3)
                        nc.tensor.matmul(px[:], atp, xcur[:], start=True, stop=True)
                        xnew = xp.tile([BLK, 2 * C], bf16, tag="zx")
                        nc.vector.tensor_tensor(xnew[:], px[:], xcur[:], op=AO.add)
                        xcur = xnew
                    # xcur = [u_loc | W~] time-major

                    # ---- W~T via PE transpose ----
                    ptr = psp.tile([C, BLK], bf16, tag="pmid", bufs=3)
                    nc.tensor.transpose(ptr[:], xcur[:, C : 2 * C], idt[:])
                    wtt = smp.tile([C, BLK], bf16, tag="wtt")
                    nc.scalar.copy(wtt[:], ptr[:])

                    # ---- U_full buffer ----
                    ufull = xp.tile([BLK, C], bf16, tag="uf")
                    nc.vector.memset(ufull[:], 0.0)

                    yb = yp.tile([C, BLK], f32, tag="yb")

                    # ---- chunk scan ----
                    for c in range(CH):
                        rc = slice(DT * c, DT * (c + 1))
                        st = sts[cur]
                        # u = u_loc + W~ @ St
                        pu = psp.tile([BLK, C], f32, tag="psml", bufs=2)
                        nc.tensor.matmul(
                            pu[rc, :],
                            wtt[:, rc],
                            st[:],
                            start=True,
                            stop=True,
                            tile_position=(0, DT * c),
                        )
                        nc.vector.tensor_tensor(
                            ufull[rc, :], pu[rc, :], xcur[rc, 0:C], op=AO.add
                        )
                        # yT = vT@qkT + uT@qbT + StT@wqT
                        py = psp.tile([C, DT], f32, tag="psml", bufs=2)
                        nc.tensor.matmul(
                            py[:], tvt[:, tm], g1[:, DT * c : DT * (c + 1)], start=True, stop=False
                        )
                        nc.tensor.matmul(
                            py[:], ufull[:], g2[:, DT * c : DT * (c + 1)], start=False, stop=False
                        )
                        nc.tensor.matmul(
                            py[:],
                            st[:],
                            qa[:, t0 + DT * c : t0 + DT * (c + 1)],
                            start=False,
                            stop=True,
                        )
                        nc.vector.tensor_copy(yb[:, rc], py[:])
                        # state update
                        ps = psp.tile([C, C], f32, tag="psml", bufs=2)
                        nc.tensor.matmul(
                            ps[:],
                            tkft[rc, tm],
                            tvt[rc, tm],
                            start=True,
                            stop=False,
                            tile_position=(DT * c, 0),
                        )
                        nc.tensor.matmul(
                            ps[:],
                            tbft[rc, tm],
                            ufull[rc, :],
                            start=False,
                            stop=True,
                            tile_position=(DT * c, 0),
                        )
                        stn = sts[1 - cur]
                        ci = n * CH + c
                        nc.vector.scalar_tensor_tensor(
                            stn[:],
                            st[:],
                            fwt[:, ci : ci + 1],
                            ps[:],
                            op0=AO.mult,
                            op1=AO.add,
                        )
                        cur = 1 - cur

                    nc.sync.dma_start(y[h, :, t0 : t0 + BLK], yb[:])
    nc.compile()
    return nc


def _host_prep(w, q, k, v, a, b):
    """Split to [BH,T,C], compute decay streams at DT=32, build per-core input maps."""
    def split(x):
        return (
            np.ascontiguousarray(x)
            .reshape(B, T, H, C)
            .transpose(0, 2, 1, 3)
            .reshape(BH, T, C)
        )

    ws, qs, ks, vs, az, bz = (split(x) for x in (w, q, k, v, a, b))
    # decay quantities per DT-chunk
    wr = ws.reshape(BH, NT, DT, C)
    dec = np.exp(-np.exp(wr))
    incl = np.cumprod(dec, axis=2)
    fw = incl[:, :, -1, :]                       # [BH, NT, C]
    non_incl = incl / dec
    inv_incl = 1.0 / incl
    r4 = lambda x: x.reshape(BH, NT, DT, C)
    wq = (r4(qs) * incl).reshape(BH, T, C)
    wa = (r4(az) * non_incl).reshape(BH, T, C)
    kwi = (r4(ks) * inv_incl).reshape(BH, T, C)
    bwi = (r4(bz) * inv_incl).reshape(BH, T, C)
    kwif = (r4(ks) * inv_incl * fw[:, :, None, :]).reshape(BH, T, C)
    bwif = (r4(bz) * inv_incl * fw[:, :, None, :]).reshape(BH, T, C)

    def cmajor(x):  # [BH,T,C] -> [NCORES, NH, C, T] bf16
        import ml_dtypes
        xt = np.ascontiguousarray(x.transpose(0, 2, 1)).astype(ml_dtypes.bfloat16)
        return xt.reshape(NCORES, NH, C, T)

    def tmajor(x):  # [BH,T,C] -> [NCORES, NH, 128, NB*C] bf16 (pre-tiled)
        import ml_dtypes
        xt = (
            x.reshape(BH, NB, BLK, C)
            .transpose(0, 2, 1, 3)
            .reshape(BH, BLK, NB * C)
            .astype(ml_dtypes.bfloat16)
        )
        return np.ascontiguousarray(xt).reshape(NCORES, NH, BLK, NB * C)

    maskp, m1 = _build_masks()
    import ml_dtypes
    bf = ml_dtypes.bfloat16
    consts = {
        "maskp": maskp.astype(bf),
        "m1c": m1.astype(bf),
        "ident": np.eye(BLK, dtype=np.float32).astype(bf),
    }
    CWQ, CWA, CKW, CBW = cmajor(wq), cmajor(wa), cmajor(kwi), cmajor(bwi)
    TV, TWA, TKWF, TBWF = tmajor(vs), tmajor(wa), tmajor(kwif), tmajor(bwif)
    FWD = np.ascontiguousarray(
        fw.transpose(0, 2, 1).astype(np.float32).reshape(NCORES, NH, C, NT)
    )
    in_maps = []
    for ci in range(NCORES):
        in_maps.append(
            dict(
                cwq=CWQ[ci], cwa=CWA[ci], ckw=CKW[ci], cbw=CBW[ci],
                tv=TV[ci], twa=TWA[ci], tkwf=TKWF[ci], tbwf=TBWF[ci],
                fwd=FWD[ci], **consts,
            )
        )
    return in_maps


def kernel(w, q, k, v, a, b):
    from concourse.bass_utils import run_bass_kernel_spmd

    if "nc" not in _CACHED:
        _CACHED["nc"] = _build_nc()
    nc = _CACHED["nc"]
    in_maps = _host_prep(w, q, k, v, a, b)
    _CACHED["in_maps"] = in_maps
    trace = bool(int(os.environ.get("RWKV_TRACE", "0")))
    res = run_bass_kernel_spmd(nc, in_maps, core_ids=list(range(NCORES)), trace=trace)
    _CACHED["last_result"] = res
    ys = np.stack([r["y"] for r in res.results])  # [NCORES, NH, C, T] f32
    yfull = ys.reshape(BH, C, T).transpose(0, 2, 1)  # [BH, T, C]
    out = (
        yfull.reshape(B, H, T, C).transpose(0, 2, 1, 3).reshape(B, T, H * C)
    )
    return np.ascontiguousarray(out.astype(np.float32))
